# revision 25
# baseline (speedup 1.0000x reference)
"""KANConv2d Trainium2 kernel (8-core data-parallel over batch).

Math: with u = (x+2.2)/0.4 clamped to [0, 11], the efficient-kan cubic
B-spline layer equals a 3x3 conv over 12 groups of 64 channels:
  [silu(x); D2_m(u) for m=0..10],   D2_m = p_m - 2 p_{m+1} + p_{m+2},
  p_m = relu(u - m)^3  (p_{>=11} = 0 by the clamp).
The second differencing bounds |D2| <= ~60 (vs ~7000 for raw truncated
powers), which makes single-pass fp16 matmuls accurate (rel err ~5e-3);
host weights are the double-prefix-sum transform of the T-folded spline
weights. The conv runs as 6 k-tiles x 9 taps x 7 psum chunks of fp16
matmuls over a (58-wide zero-padded, flattened) feature map.
"""
import math
import numpy as np

import concourse.bass as bass
import concourse.mybir as mybir
from concourse.tile import TileContext

# ---- problem constants (hardcoded per harness contract) ----
B, C, H, W = 8, 64, 56, 56
OC = 128
GRID_SIZE, SPLINE_ORDER = 5, 3
HSTEP = 0.4
NM = 12                                        # truncated powers m=0..11
NCH_GROUPS = 12                                # silu + D2_0..D2_10
WP = W + 2                                     # 58 padded width
PADFLAT = WP * WP + 4                          # 3368
NCHUNK = 464                                   # 8 rows * 58 (<=512 psum fp32)
NCH = 7                                        # chunks: 7*464 = 3248
NKT = 6                                        # k-tiles per tap
F32 = mybir.dt.float32
F16 = mybir.dt.float16


def _patch_tile_drain():
    """walrus in this container rejects sem waits on InstDrain (CTRL_NO
    struct): move the end-of-kernel drain waits onto single-wait NOPs."""
    import bass_rust

    def _drain_and_barrier(self, tick_clock, wait_clock):
        collector = self.nc.sync.nop(nofuse=True, hint="drain_waits")
        wait_clock.add_sem_waits(
            collector.ins, bass_rust.ScopedClock({None: tick_clock.global_clock})
        )
        waits = list(collector.ins.sync_info.on_wait)
        collector.ins.sync_info = mybir.SyncInfo(on_wait=waits[:1], on_update=[])
        for w in waits[1:]:
            n = self.nc.sync.nop(nofuse=True, hint="drain_waits")
            n.ins.sync_info = mybir.SyncInfo(on_wait=[w], on_update=[])
        self.nc.sync.drain()
        self.nc.all_engine_barrier()
        popped = self.nc._tile_sem_poison_stack.pop()
        assert popped is self._sem_poison
        self.nc.clear_and_free_semaphores(list(self.sems.allocated().values()))
        self.nc.all_engine_barrier()

    TileContext._drain_and_barrier = _drain_and_barrier


_patch_tile_drain()


def _split_excess_waits(nc):
    """This walrus caps sync waits at 1/instruction (2 for EventSemaphore).
    Spill excess waits onto EventSemaphore insts inserted just before the
    overloaded instruction on the same engine."""
    import bass_rust

    counter = [0]
    for func in nc.m.functions:
        for bb in func.blocks:
            insts = bb.instructions
            out = []
            changed = False
            for inst in insts:
                si = getattr(inst, "sync_info", None)
                waits = list(si.on_wait) if si is not None else []
                cap = 2 if isinstance(inst, bass_rust.InstEventSemaphore) else 1
                if len(waits) > cap:
                    excess = waits[cap:]
                    for i in range(0, len(excess), 2):
                        counter[0] += 1
                        ev = bass_rust.InstEventSemaphore(
                            name=f"evspill-{counter[0]}",
                            engine=inst.engine,
                            ins=[], outs=[],
                            sync_info=mybir.SyncInfo(
                                on_wait=excess[i:i + 2], on_update=[]),
                        )
                        out.append(ev)
                    inst.sync_info = mybir.SyncInfo(
                        on_wait=waits[:cap], on_update=list(si.on_update))
                    changed = True
                out.append(inst)
            if changed:
                bb.instructions = out


def _host_weights(base_weight, spline_weight, spline_scaler):
    """Fold spline bases into D2-channel conv weights.

    Returns wt[128, 54*128] fp16 (row, j=b*9+s, o) and bias[128, 8] fp32.
    """
    T = np.zeros((8, NM), dtype=np.float64)
    for g in range(8):
        for r in range(SPLINE_ORDER + 2):
            T[g, g + r] = ((-1) ** r) * math.comb(SPLINE_ORDER + 1, r) / 6.0
    scaled = spline_weight.astype(np.float64) * spline_scaler.astype(np.float64)[..., None]
    W2 = np.einsum("oig,gm->oim", scaled, T)        # (O, 576, 12)
    # W''_j = W2_j + 2 W''_{j-1} - W''_{j-2}  (D2 un-differencing transform)
    Wd = np.zeros((OC, 9 * C, 11), dtype=np.float64)
    for j in range(11):
        Wd[..., j] = W2[..., j]
        if j >= 1:
            Wd[..., j] += 2.0 * Wd[..., j - 1]
        if j >= 2:
            Wd[..., j] -= Wd[..., j - 2]
    bw = base_weight.astype(np.float64)             # (O, 576)

    # tile groups: b=0: (D2_0 | silu); b=1..5: (D2_{2b-1} | D2_{2b})
    wt = np.zeros((128, 54, OC), dtype=np.float16)
    for b in range(NKT):
        m_lo = 0 if b == 0 else 2 * b - 1
        for s in range(9):
            kh, kw = s // 3, s % 3
            j = b * 9 + s
            for half, m in ((0, m_lo), (1, None if b == 0 else 2 * b)):
                rows = slice(64 * half, 64 * half + 64)
                c = np.arange(C)
                i = c * 9 + kh * 3 + kw
                if b == 0 and half == 1:
                    wt[rows, j, :] = bw[:, i].T.astype(np.float16)
                else:
                    wt[rows, j, :] = Wd[:, i, m].T.astype(np.float16)
    wt = wt.reshape(128, 54 * OC)

    bias = np.zeros((128, 8), dtype=np.float32)
    for b in range(NKT):
        bias[0:64, b] = 11.0 - 2 * b
        bias[64:128, b] = 11.0 - (2 * b + 1)
    bias[:, 6] = 5.5
    return wt, bias


def _build_nc():
    nc = bass.Bass()
    x_in = nc.declare_dram_parameter("x", [C, H, W], F32, isOutput=False)
    wt_in = nc.declare_dram_parameter("wt", [128, 54 * OC], F16, isOutput=False)
    bias_in = nc.declare_dram_parameter("bias", [128, 8], F32, isOutput=False)
    out = nc.declare_dram_parameter("out", [OC, H, W], F32, isOutput=True)

    AF = mybir.ActivationFunctionType
    ALU = mybir.AluOpType

    with TileContext(nc) as tc:
        with (
            tc.tile_pool(name="w", bufs=1) as wpool,
            tc.tile_pool(name="xf", bufs=1) as xfpool,
            tc.tile_pool(name="scr", bufs=4) as scrpool,
            tc.tile_pool(name="cub", bufs=3) as cubpool,
            tc.tile_pool(name="xsh", bufs=3) as xshpool,
            tc.tile_pool(name="dt", bufs=3) as dpool,
            tc.tile_pool(name="ob", bufs=7) as obpool,
            tc.tile_pool(name="psum", bufs=1, space="PSUM") as psumpool,
        ):
            bias_sb = wpool.tile([128, 8], F32, tag="bias_sb")
            nc.sync.dma_start(bias_sb[:], bias_in[:])

            HALF = PADFLAT // 2  # 1684; column-split halves latency of prep ops
            COLS = [(0, HALF), (HALF, PADFLAT)]

            xpad = xfpool.tile([128, PADFLAT], F32, tag="xpad")
            xv = xpad[:, :WP * WP].rearrange("p (r c) -> p r c", c=WP)
            # zero only the pad border so the interior DMAs don't wait on a
            # full-tile memset: row 0, row 57 (+tail), cols 0 and 57
            nc.gpsimd.memset(xv[:, 0, :], 0.0)
            nc.gpsimd.memset(xpad[:, 57 * WP:PADFLAT], 0.0)
            nc.gpsimd.memset(xv[:, 1:57, 0], 0.0)
            nc.gpsimd.memset(xv[:, 1:57, 57], 0.0)
            # quarter-row DMAs on the fast sync/scalar queues
            dma_engines = [nc.sync, nc.scalar, nc.sync, nc.scalar]
            for q in range(4):
                r0, r1 = 1 + 14 * q, 1 + min(14 * (q + 1), H)
                eng = dma_engines[q]
                eng.dma_start(xv[0:C, r0:r1, 1:W + 1], x_in[:, r0 - 1:r1 - 1, :])
                eng.dma_start(xv[64:64 + C, r0:r1, 1:W + 1], x_in[:, r0 - 1:r1 - 1, :])

            # b=0 weight block in its OWN tile: tile-granular dependency
            # tracking means the warmup/silu-strip matmuls would otherwise
            # wait for the whole 1.8MB weight transfer
            w0_sb = wpool.tile([128, 9 * OC], F16, tag="w0_sb")
            nc.sync.dma_start(w0_sb[:], wt_in[:, 0:9 * OC])
            w1_sb = wpool.tile([128, 45 * OC], F16, tag="w1_sb")
            nc.sync.dma_start(w1_sb[:], wt_in[:, 9 * OC:54 * OC])

            # preload the ACT function table while input DMAs run
            dummy = wpool.tile([128, 1], F32, tag="dummy")
            nc.scalar.activation(dummy[:], bias_sb[:, 0:1], AF.Relu,
                                 bias=bias_sb[:, 7:8])

            # HAM warmup: junk matmuls into the unused 8th PSUM bank keep the
            # PE busy so the clock gate flips to 8/8 before the real stream
            # (and stays there across the silu->D2_0 pipeline-fill gap)
            jp = psumpool.tile([128, 464], F32, tag="jp", name="jp")

            def junk_mms(n, base):
                # operands stay inside the first 9*OC cols (the early w block)
                for i in range(n):
                    half = jp[:, 0:232] if i % 2 == 0 else jp[:, 232:464]
                    off = ((base + i) * 101) % (9 * OC - 232)
                    nc.tensor.matmul(half, w0_sb[:, 0:OC], w0_sb[:, off:off + 232],
                                     start=True, stop=True)

            junk_mms(24, 0)

            # t = relu(5.5 - 2.5 x) = 11 - min(u, 11)
            tmap = xfpool.tile([128, PADFLAT], F32, tag="tmap")
            for lo, hi in COLS:
                nc.scalar.activation(tmap[:, lo:hi], xpad[:, lo:hi], AF.Relu,
                                     scale=-2.5, bias=bias_sb[:, 6:7])

            psum = [psumpool.tile([128, NCHUNK], F32, tag=f"pb{k}", name=f"pb{k}")
                    for k in range(NCH)]

            def cube_tile(b):
                # c_b = relu((11-m) - t)^3, m = (2b | 2b+1) per half
                r = scrpool.tile([128, PADFLAT], F32, tag="scr", name=f"r{b}")
                s = scrpool.tile([128, PADFLAT], F32, tag="scr", name=f"s{b}")
                c = cubpool.tile([128, PADFLAT], F32, tag="cub", name=f"c{b}")
                for lo, hi in COLS:
                    nc.scalar.activation(r[:, lo:hi], tmap[:, lo:hi], AF.Relu,
                                         scale=-1.0, bias=bias_sb[:, b:b + 1])
                    if b < 2:
                        # pipeline-fill critical path: square on DVE
                        nc.vector.tensor_mul(s[:, lo:hi], r[:, lo:hi], r[:, lo:hi])
                    else:
                        # s = v^2 (sign-free: r==0 kills the v<0 region anyway)
                        nc.scalar.activation(s[:, lo:hi], tmap[:, lo:hi], AF.Square,
                                             scale=-1.0, bias=bias_sb[:, b:b + 1])
                    eng = nc.vector if b < 2 else nc.gpsimd
                    eng.tensor_mul(c[:, lo:hi], s[:, lo:hi], r[:, lo:hi])
                return c

            def x_tile(b, c_lo, c_hi):
                # X_b = (c_b upper | c_{b+1} lower) = (p_{2b+1} | p_{2b+2})
                xt = xshpool.tile([128, PADFLAT], F32, tag="xsh", name=f"x{b}")
                for lo, hi in COLS:
                    nc.sync.dma_start(xt[0:64, lo:hi], c_lo[64:128, lo:hi])
                    nc.sync.dma_start(xt[64:128, lo:hi], c_hi[0:64, lo:hi])
                return xt

            def run_matmuls(b, dt_tile, first, last):
                for s in range(9):
                    kh, kw = s // 3, s % 3
                    off = kh * WP + kw
                    lhsT = w1_sb[:, ((b - 1) * 9 + s) * OC:((b - 1) * 9 + s + 1) * OC]
                    for k in range(NCH):
                        rhs = dt_tile[:, off + k * NCHUNK: off + k * NCHUNK + NCHUNK]
                        nc.tensor.matmul(psum[k][:], lhsT, rhs,
                                         start=(first and s == 0),
                                         stop=(last and s == 8))

            def run_strip_matmuls(dt_tile, pl, ph, first):
                # K=64 row-strip matmuls for the b=0 k-tile (silu | D2_0):
                # lets the silu half start ~25us before the D2_0 half is ready
                for s in range(9):
                    kh, kw = s // 3, s % 3
                    off = kh * WP + kw
                    lhsT = w0_sb[pl:ph, s * OC:(s + 1) * OC]
                    for k in range(NCH):
                        rhs = dt_tile[pl:ph, off + k * NCHUNK: off + k * NCHUNK + NCHUNK]
                        nc.tensor.matmul(psum[k][:], lhsT, rhs,
                                         start=(first and s == 0), stop=False)

            # stage pipeline over b
            cubes = {}
            xts = {}
            # tmap emitted above; silu next: it is half of D tile 0
            d0 = dpool.tile([128, PADFLAT], F16, tag="dt", name="d0")
            for lo, hi in COLS:
                nc.scalar.activation(d0[64:128, lo:hi], xpad[64:128, lo:hi], AF.Silu)
            run_strip_matmuls(d0, 64, 128, first=True)
            junk_mms(75, 24)

            cubes[0] = cube_tile(0)
            cubes[1] = cube_tile(1)
            xts[0] = x_tile(0, cubes[0], cubes[1])

            # D tile 0 lower = D2_0 = c_0 - 2 X_0 + c_1 (lower halves)
            tmp0 = scrpool.tile([128, PADFLAT], F32, tag="scr", name="tmp0")
            for lo, hi in COLS:
                nc.vector.tensor_add(tmp0[0:64, lo:hi], cubes[0][0:64, lo:hi],
                                     cubes[1][0:64, lo:hi])
                nc.vector.scalar_tensor_tensor(
                    d0[0:64, lo:hi], xts[0][0:64, lo:hi], -2.0, tmp0[0:64, lo:hi],
                    ALU.mult, ALU.add)
            run_strip_matmuls(d0, 0, 64, first=False)

            for bp in range(1, 6):
                # D tile bp = X_{bp-1} - 2 c_bp + X_bp  (X_5 == 0)
                if bp + 1 <= 5:
                    cubes[bp + 1] = cube_tile(bp + 1)
                    xts[bp] = x_tile(bp, cubes[bp], cubes[bp + 1])
                dt = dpool.tile([128, PADFLAT], F16, tag="dt", name=f"d{bp}")
                if bp < 5:
                    tmp = scrpool.tile([128, PADFLAT], F32, tag="scr", name=f"tmp{bp}")
                    for lo, hi in COLS:
                        nc.vector.tensor_add(tmp[:, lo:hi], xts[bp - 1][:, lo:hi],
                                             xts[bp][:, lo:hi])
                        nc.vector.scalar_tensor_tensor(
                            dt[:, lo:hi], cubes[bp][:, lo:hi], -2.0, tmp[:, lo:hi],
                            ALU.mult, ALU.add)
                else:
                    for lo, hi in COLS:
                        nc.vector.scalar_tensor_tensor(
                            dt[:, lo:hi], cubes[5][:, lo:hi], -2.0,
                            xts[4][:, lo:hi], ALU.mult, ALU.add)
                if bp < 5:
                    run_matmuls(bp, dt, first=False, last=False)
                else:
                    # last stage chunk-major: chunk k's accumulation completes
                    # after its 9 taps, so its copy+output DMA overlaps the
                    # remaining chunks' matmuls
                    for k in range(NCH):
                        for s in range(9):
                            kh, kw = s // 3, s % 3
                            off = kh * WP + kw
                            lhsT = w1_sb[:, (36 + s) * OC:(36 + s + 1) * OC]
                            rhs = dt[:, off + k * NCHUNK: off + k * NCHUNK + NCHUNK]
                            nc.tensor.matmul(psum[k][:], lhsT, rhs,
                                             start=False, stop=(s == 8))
                        ob = obpool.tile([128, NCHUNK], F32, tag="ob", name=f"ob{k}")
                        nc.scalar.activation(ob[:], psum[k][:], AF.Copy)
                        src = ob[:].rearrange("p (r c) -> p r c", c=WP)[:, :, 0:W]
                        nc.sync.dma_start(out[:, 8 * k:8 * k + 8, :], src)
    _split_excess_waits(nc)
    return nc


_CACHE = {}


def kernel(x, base_weight, spline_weight, spline_scaler):
    from concourse.bass_utils import run_bass_kernel_spmd

    x = np.ascontiguousarray(x, dtype=np.float32)
    wt, bias = _host_weights(
        np.asarray(base_weight, np.float32),
        np.asarray(spline_weight, np.float32),
        np.asarray(spline_scaler, np.float32),
    )
    if "nc" not in _CACHE:
        _CACHE["nc"] = _build_nc()
    nc = _CACHE["nc"]
    in_maps = [{"x": x[b], "wt": wt, "bias": bias} for b in range(B)]
    res = run_bass_kernel_spmd(nc, in_maps, list(range(B)))
    out = np.stack([res.results[b]["out"] for b in range(B)], axis=0)
    return out
